# revision 17
# baseline (speedup 1.0000x reference)
"""Chamfer loss kernel for Trainium2 (8 NeuronCores, SPMD).

Math: for each batch b, d2[m,n] = ||t[m]-a[n]||^2 over 2D points.
  loss = mean_{b,m} min_n d[m,n] + mean_{b,n} min_m d[m,n]

Strategy per core (2 batches/core, data-parallel over batch):
  - P[m,n] = t.a - a2[n]/2 computed on the TensorEngine, so
    min_n d2 = t2[m] - 2*max_n P[m,n]  (t2 applied per-partition afterwards).
  - fp32 matmuls run at 4 cycles/row on TRN2; instead each value is split
    into THREE bf16 terms (t = t1+t2+t3, a = a1+a2+a3, s2 = q1+q2+q3) and
    P is ONE K=15 bf16 matmul (1 cycle/row) keeping all cross products
    t_i.a_j with i+j <= 4, giving ~1e-6 absolute d2 error (2-term splits
    leave ~1e-4 tails, the same order as d2min itself).
  - Row-max of each [128,2048] PSUM tile via DVE reduce_max (the fused
    tensor_tensor_reduce ISA op crashes this runtime).
  - Backward direction = same kernel with roles swapped (second pass).
  - Finalize: d = sqrt(relu(-2*max + s2col)), sum on-chip, PE ones-matmul
    partition-sum, one scalar out per core; host sums 8 scalars and divides.

Groups g = 2*b + s, s=0: fwd (self=target, opp=actual), s=1: bwd (swapped).
Group g's matmul operands live at partitions 32g..32g+8 (PE row-group g).
"""

import numpy as np

B, M, N = 16, 2048, 2048
NCORES = 8
BPC = B // NCORES  # batches per core
NG = 2 * BPC  # groups per core: (fwd,bwd) x batches
MT = M // 128  # m-tiles per group

_CACHE = {}


def _build_program():
    from concourse import bacc, mybir

    fp32 = mybir.dt.float32
    bf16 = mybir.dt.bfloat16
    Alu = mybir.AluOpType
    import concourse.tile as tile

    nc = bacc.Bacc("TRN2", target_bir_lowering=False, debug=False)
    tgt = nc.dram_tensor("tgt", [BPC, M, 2], fp32, kind="ExternalInput").ap()
    act = nc.dram_tensor("act", [BPC, M, 2], fp32, kind="ExternalInput").ap()
    out = nc.dram_tensor("out", [1, 1], fp32, kind="ExternalOutput").ap()

    with tile.TileContext(nc) as tc:
        with (
            tc.tile_pool(name="singles", bufs=1) as singles,
            tc.tile_pool(name="scr", bufs=3) as scr_pool,
            tc.tile_pool(name="psum", bufs=2, space="PSUM") as psum_pool,
        ):
            DUM = singles.tile([1, 1], fp32, tag="DUM")
            nc.gpsimd.memset(DUM[:], 1.0)
            nc.scalar.square(DUM[:], DUM[:])
            # bf16 matmul operand tiles; group g occupies rows 32g..32g+14.
            # K=15 band pairing (lhsT row . rhs row):
            #   L1x.A1x L1y.A1y | L1x.A2x L1y.A2y | L2x.A1x L2y.A1y
            #   L1x.A3x L1y.A3y | L3x.A1x L3y.A1y | L2x.A2x L2y.A2y
            #   c.q1 c.q2 c.q3   (c = -1/2, q_i = s2 splits)
            L16 = singles.tile([128, M], bf16, tag="L16")
            R16 = singles.tile([128, M], bf16, tag="R16")
            # fp32 staging rows 3g+{0,1,2} = [x_opp, y_opp, s2_opp], base 0
            RCALL = singles.tile([3 * NG, M], fp32, tag="RCALL")
            BAA = singles.tile([3 * NG, M], bf16, tag="BAA")  # term 1
            BBA = singles.tile([3 * NG, M], bf16, tag="BBA")  # term 2
            BCA = singles.tile([3 * NG, M], bf16, tag="BCA")  # term 3
            R1A = singles.tile([3 * NG, M], fp32, tag="R1A")
            R2A = singles.tile([3 * NG, M], fp32, tag="R2A")
            # opp coords for squares (x block, y block), contiguous base 0
            XYX = singles.tile([NG, M], fp32, tag="XYX")
            XYY = singles.tile([NG, M], fp32, tag="XYY")
            SQX = singles.tile([NG, M], fp32, tag="SQX")
            SQY = singles.tile([NG, M], fp32, tag="SQY")
            S2 = singles.tile([NG, M], fp32, tag="S2")
            TP = singles.tile([128, NG * 2 * MT], fp32, tag="TP")
            SQTP = singles.tile([128, NG * 2 * MT], fp32, tag="SQTP")
            CS = singles.tile([128, NG * MT], fp32, tag="CS")
            MR = singles.tile([128, NG * MT], fp32, tag="MR")
            U = singles.tile([128, NG * MT], fp32, tag="U")
            D = singles.tile([128, NG * MT], fp32, tag="D")
            SUM = singles.tile([128, 1], fp32, tag="SUM")
            CROWC3 = singles.tile([3, M], bf16, tag="CROWC3")

            # ---- input staging ----
            for b in range(BPC):
                for s, (self_t, opp_t) in enumerate(((tgt, act), (act, tgt))):
                    g = 2 * b + s
                    om = opp_t[b].rearrange("m c -> c m")  # [2, M] coord-major
                    eng = nc.sync if g % 2 == 0 else nc.scalar
                    eng.dma_start(RCALL[3 * g : 3 * g + 2, :], om)
                    nc.scalar.dma_start(XYX[g : g + 1, :], RCALL[3 * g : 3 * g + 1, :])
                    nc.scalar.dma_start(
                        XYY[g : g + 1, :], RCALL[3 * g + 1 : 3 * g + 2, :]
                    )

            # fp32 prelude operands for group 0's first units: lhsT rows
            # [x_T0, y_T0, -1/2] (exact fp32, no split-chain dependency)
            LF = singles.tile([3, M], fp32, tag="LF")
            CROWCF = singles.tile([1, M], fp32, tag="CROWCF")
            nc.gpsimd.memset(CROWCF[:], -0.5)
            nc.sync.dma_start(LF[0:2, :], RCALL[3:5, :])
            nc.sync.dma_start(LF[2:3, :], CROWCF[:])

            nc.gpsimd.memset(CROWC3[:], -0.5)
            # opp squared norms: S2[g] = x^2 + y^2
            nc.scalar.square(SQX[:], XYX[:])
            nc.vector.tensor_mul(SQY[:], XYY[:], XYY[:])
            nc.vector.tensor_add(S2[:], SQX[:], SQY[:])
            for g in range(NG):
                eng = nc.sync if g % 2 == 0 else nc.scalar
                eng.dma_start(RCALL[3 * g + 2 : 3 * g + 3, :], S2[g : g + 1, :])

            # three-term bf16 split of all staging rows
            with tc.high_priority():
                nc.scalar.copy(BAA[:], RCALL[:])
                nc.vector.tensor_tensor(R1A[:], RCALL[:], BAA[:], op=Alu.subtract)
                nc.scalar.copy(BBA[:], R1A[:])
                nc.vector.tensor_tensor(R2A[:], R1A[:], BBA[:], op=Alu.subtract)
                nc.scalar.copy(BCA[:], R2A[:])

            # self coords partition-major for the t2 columns (finalize-only;
            # SWDGE queue; gated behind the splits via the tiny copy below so
            # its DRAM wires don't serialize ahead of the staging gathers)
            nc.gpsimd.dma_start(TP[0:1, :], BCA[0:1, 0 : NG * 2 * MT])
            for b in range(BPC):
                for s, (self_t, opp_t) in enumerate(((tgt, act), (act, tgt))):
                    g = 2 * b + s
                    nc.gpsimd.dma_start(
                        TP[:, g * 2 * MT : (g + 1) * 2 * MT].rearrange(
                            "p (i c) -> p i c", c=2
                        ),
                        self_t[b].rearrange("(i p) c -> p i c", p=128),
                    )
            nc.vector.tensor_mul(SQTP[:], TP[:], TP[:])
            SQv = SQTP[:].rearrange("p (k c) -> p k c", c=2)
            nc.vector.tensor_add(CS[:], SQv[:, :, 0], SQv[:, :, 1])

            # ---- main loop: K=15 bf16 matmuls + row-max reduce ----
            # Band scatter for group g is emitted just before g's units so
            # later groups' DMAs overlap the running compute; each group's 16
            # DMAs split across both HWDGE queues.
            NP = _CACHE.get("prelude", 0)  # fp32-prelude units in group 0
            for g in range(_CACHE.get("glimit", NG)):
                go = g ^ 1  # paired group: self splits of g = opp splits of go
                r = 32 * g
                c, co = 3 * g, 3 * go
                BAc, BBc, BCc = BAA, BBA, BCA
                BAo, BBo, BCo = BAA, BBA, BCA
                nc.sync.dma_start(R16[r + 0 : r + 2, :], BAc[c : c + 2, :])
                nc.sync.dma_start(R16[r + 2 : r + 4, :], BBc[c : c + 2, :])
                nc.sync.dma_start(R16[r + 4 : r + 6, :], BAc[c : c + 2, :])
                nc.sync.dma_start(R16[r + 6 : r + 8, :], BCc[c : c + 2, :])
                nc.sync.dma_start(R16[r + 8 : r + 10, :], BAc[c : c + 2, :])
                nc.sync.dma_start(R16[r + 10 : r + 12, :], BBc[c : c + 2, :])
                nc.sync.dma_start(R16[r + 12 : r + 13, :], BAc[c + 2 : c + 3, :])
                nc.sync.dma_start(R16[r + 13 : r + 14, :], BBc[c + 2 : c + 3, :])
                nc.sync.dma_start(R16[r + 14 : r + 15, :], BCc[c + 2 : c + 3, :])
                nc.sync.dma_start(L16[r + 0 : r + 2, :], BAo[co : co + 2, :])
                nc.sync.dma_start(L16[r + 2 : r + 4, :], BAo[co : co + 2, :])
                nc.sync.dma_start(L16[r + 4 : r + 6, :], BBo[co : co + 2, :])
                nc.sync.dma_start(L16[r + 6 : r + 8, :], BAo[co : co + 2, :])
                nc.sync.dma_start(L16[r + 8 : r + 10, :], BCo[co : co + 2, :])
                nc.sync.dma_start(L16[r + 10 : r + 12, :], BBo[co : co + 2, :])
                nc.sync.dma_start(L16[r + 12 : r + 15, :], CROWC3[:])
                lhsT = L16[32 * g : 32 * g + 15, :]
                rhs = R16[32 * g : 32 * g + 15, :]
                for i in range(_CACHE.get('mtlimit', MT)):
                    if g == 0 and i < NP:
                        lhsT_u, rhs_u = LF[:], RCALL[0:3, :]
                    else:
                        lhsT_u, rhs_u = lhsT, rhs
                    # Split PSUM halves into separate slot pools: the upper
                    # half recycles right after the ACT copy, so matmuls run
                    # two units ahead and the DVE scan stays back-to-back.
                    PU = psum_pool.tile([128, N // 2], fp32, tag="PU", bufs=2)
                    PL = psum_pool.tile([128, N // 2], fp32, tag="PL", bufs=2)
                    for j in range(2):
                        nc.tensor.matmul(
                            PU[:, 512 * j : 512 * (j + 1)],
                            lhsT_u[:, 128 * i : 128 * (i + 1)],
                            rhs_u[:, 512 * (j + 2) : 512 * (j + 3)],
                            start=True,
                            stop=True,
                            tile_position=(32 * g, 0),
                        )
                    # row-max via scan: ACT stages the upper half in SBUF so
                    # the DVE scan ingests 2 elems/cycle (PSUM + SBUF); the
                    # running max lands in the MR column via broadcast-out.
                    half = scr_pool.tile([128, N // 2], fp32, tag="half")
                    nc.scalar.copy(half[:], PU[:])
                    for j in range(2):
                        nc.tensor.matmul(
                            PL[:, 512 * j : 512 * (j + 1)],
                            lhsT_u[:, 128 * i : 128 * (i + 1)],
                            rhs_u[:, 512 * j : 512 * (j + 1)],
                            start=True,
                            stop=True,
                            tile_position=(32 * g, 0),
                        )
                    nc.vector.tensor_tensor_scan(
                        MR[:, g * MT + i : g * MT + i + 1].broadcast_to(
                            (128, N // 2)
                        ),
                        PL[:],
                        half[:],
                        initial=-3.0e38,
                        op0=Alu.max,
                        op1=Alu.max,
                    )

            # ---- finalize: d2min = CS - 2*MR ; d = sqrt(relu(d2min)) ----
            nc.vector.scalar_tensor_tensor(
                U[:], MR[:], -2.0, CS[:], op0=Alu.mult, op1=Alu.add
            )
            nc.vector.tensor_scalar_max(U[:], U[:], 0.0)
            nc.scalar.sqrt(D[:], U[:])
            nc.vector.reduce_sum(SUM[:], D[:], axis=mybir.AxisListType.X)
            # partition sum via PE: [1,1] = SUM.T @ ones
            ONES = singles.tile([128, 1], fp32, tag="ONES")
            OUTS = singles.tile([1, 1], fp32, tag="OUTS")
            nc.gpsimd.memset(ONES[:], 1.0)
            acc = psum_pool.tile([1, 1], fp32, tag="PL", bufs=2)
            nc.tensor.matmul(acc[:], SUM[:], ONES[:], start=True, stop=True)
            nc.scalar.copy(OUTS[:], acc[:])
            nc.sync.dma_start(out, OUTS[:])

    nc.compile()
    return nc


def _get_program():
    if "nc" not in _CACHE:
        _CACHE["nc"] = _build_program()
    return _CACHE["nc"]


def kernel(target_points, actual_points):
    tgt = np.ascontiguousarray(np.asarray(target_points, dtype=np.float32))
    act = np.ascontiguousarray(np.asarray(actual_points, dtype=np.float32))
    assert tgt.shape == (B, M, 2) and act.shape == (B, N, 2)

    nc = _get_program()
    in_maps = [
        {"tgt": tgt[c * BPC : (c + 1) * BPC], "act": act[c * BPC : (c + 1) * BPC]}
        for c in range(NCORES)
    ]
    from concourse import bass_utils

    res = bass_utils.run_bass_kernel_spmd(nc, in_maps, core_ids=list(range(NCORES)))
    total = sum(float(r["out"][0, 0]) for r in res.results)
    return np.float32(total / (B * M))



# revision 19
# speedup vs baseline: 1.0302x; 1.0302x over previous
"""Chamfer loss kernel for Trainium2 (8 NeuronCores, SPMD).

Math: for each batch b, d2[m,n] = ||t[m]-a[n]||^2 over 2D points.
  loss = mean_{b,m} min_n d[m,n] + mean_{b,n} min_m d[m,n]

Strategy per core (2 batches/core, data-parallel over batch):
  - P[m,n] = t.a - a2[n]/2 computed on the TensorEngine, so
    min_n d2 = t2[m] - 2*max_n P[m,n]  (t2 applied per-partition afterwards).
  - fp32 matmuls run at 4 cycles/row on TRN2; instead each value is split
    into THREE bf16 terms (t = t1+t2+t3, a = a1+a2+a3, s2 = q1+q2+q3) and
    P is ONE K=15 bf16 matmul (1 cycle/row) keeping all cross products
    t_i.a_j with i+j <= 4, giving ~1e-6 absolute d2 error (2-term splits
    leave ~1e-4 tails, the same order as d2min itself).
  - Row-max of each [128,2048] PSUM tile via DVE reduce_max (the fused
    tensor_tensor_reduce ISA op crashes this runtime).
  - Backward direction = same kernel with roles swapped (second pass).
  - Finalize: d = sqrt(relu(-2*max + s2col)), sum on-chip, PE ones-matmul
    partition-sum, one scalar out per core; host sums 8 scalars and divides.

Groups g = 2*b + s, s=0: fwd (self=target, opp=actual), s=1: bwd (swapped).
Group g's matmul operands live at partitions 32g..32g+8 (PE row-group g).
"""

import numpy as np

B, M, N = 16, 2048, 2048
NCORES = 8
BPC = B // NCORES  # batches per core
NG = 2 * BPC  # groups per core: (fwd,bwd) x batches
MT = M // 128  # m-tiles per group

_CACHE = {}


def _build_program():
    from concourse import bacc, mybir

    fp32 = mybir.dt.float32
    bf16 = mybir.dt.bfloat16
    Alu = mybir.AluOpType
    import concourse.tile as tile

    nc = bacc.Bacc("TRN2", target_bir_lowering=False, debug=False)
    tgt = nc.dram_tensor("tgt", [BPC, M, 2], fp32, kind="ExternalInput").ap()
    act = nc.dram_tensor("act", [BPC, M, 2], fp32, kind="ExternalInput").ap()
    out = nc.dram_tensor("out", [1, 1], fp32, kind="ExternalOutput").ap()

    with tile.TileContext(nc) as tc:
        with (
            tc.tile_pool(name="singles", bufs=1) as singles,
            tc.tile_pool(name="scr", bufs=3) as scr_pool,
            tc.tile_pool(name="psum", bufs=2, space="PSUM") as psum_pool,
        ):
            DUM = singles.tile([1, 1], fp32, tag="DUM")
            nc.gpsimd.memset(DUM[:], 1.0)
            nc.scalar.square(DUM[:], DUM[:])
            # bf16 matmul operand tiles; group g occupies rows 32g..32g+14.
            # K=15 band pairing (lhsT row . rhs row):
            #   L1x.A1x L1y.A1y | L1x.A2x L1y.A2y | L2x.A1x L2y.A1y
            #   L1x.A3x L1y.A3y | L3x.A1x L3y.A1y | L2x.A2x L2y.A2y
            #   c.q1 c.q2 c.q3   (c = -1/2, q_i = s2 splits)
            L16 = singles.tile([128, M], bf16, tag="L16")
            R16 = singles.tile([128, M], bf16, tag="R16")
            # fp32 staging rows 3g+{0,1,2} = [x_opp, y_opp, s2_opp], base 0
            RCALL = singles.tile([3 * NG, M], fp32, tag="RCALL")
            BAA = singles.tile([3 * NG, M], bf16, tag="BAA")  # term 1
            BBA = singles.tile([3 * NG, M], bf16, tag="BBA")  # term 2
            BCA = singles.tile([3 * NG, M], bf16, tag="BCA")  # term 3
            R1A = singles.tile([3 * NG, M], fp32, tag="R1A")
            R2A = singles.tile([3 * NG, M], fp32, tag="R2A")
            # opp coords for squares (x block, y block), contiguous base 0
            XYX = singles.tile([NG, M], fp32, tag="XYX")
            XYY = singles.tile([NG, M], fp32, tag="XYY")
            SQX = singles.tile([NG, M], fp32, tag="SQX")
            SQY = singles.tile([NG, M], fp32, tag="SQY")
            S2 = singles.tile([NG, M], fp32, tag="S2")
            TP = singles.tile([128, NG * 2 * MT], fp32, tag="TP")
            SQTP = singles.tile([128, NG * 2 * MT], fp32, tag="SQTP")
            CS = singles.tile([128, NG * MT], fp32, tag="CS")
            MR = singles.tile([128, NG * MT], fp32, tag="MR")
            U = singles.tile([128, NG * MT], fp32, tag="U")
            D = singles.tile([128, NG * MT], fp32, tag="D")
            SUM = singles.tile([128, 1], fp32, tag="SUM")
            CROWC3 = singles.tile([3, M], bf16, tag="CROWC3")

            # ---- input staging ----
            for b in range(BPC):
                for s, (self_t, opp_t) in enumerate(((tgt, act), (act, tgt))):
                    g = 2 * b + s
                    om = opp_t[b].rearrange("m c -> c m")  # [2, M] coord-major
                    eng = nc.sync if g % 2 == 0 else nc.scalar
                    eng.dma_start(RCALL[3 * g : 3 * g + 2, :], om)
                    nc.scalar.dma_start(XYX[g : g + 1, :], RCALL[3 * g : 3 * g + 1, :])
                    nc.scalar.dma_start(
                        XYY[g : g + 1, :], RCALL[3 * g + 1 : 3 * g + 2, :]
                    )

            # fp32 prelude operands for group 0's first units: lhsT rows
            # [x_T0, y_T0, -1/2] (exact fp32, no split-chain dependency)
            LF = singles.tile([3, M], fp32, tag="LF")
            CROWCF = singles.tile([1, M], fp32, tag="CROWCF")
            nc.gpsimd.memset(CROWCF[:], -0.5)
            nc.sync.dma_start(LF[0:2, :], RCALL[3:5, :])
            nc.sync.dma_start(LF[2:3, :], CROWCF[:])

            nc.gpsimd.memset(CROWC3[:], -0.5)
            # opp squared norms: S2[g] = x^2 + y^2, by column halves so the
            # split chain's first half unblocks sooner
            for h in range(2):
                hs = slice(h * (M // 2), (h + 1) * (M // 2))
                nc.scalar.square(SQX[:, hs], XYX[:, hs])
                nc.vector.tensor_mul(SQY[:, hs], XYY[:, hs], XYY[:, hs])
                nc.vector.tensor_add(S2[:, hs], SQX[:, hs], SQY[:, hs])
                for g in range(NG):
                    eng = nc.sync if g % 2 == 0 else nc.scalar
                    eng.dma_start(
                        RCALL[3 * g + 2 : 3 * g + 3, hs], S2[g : g + 1, hs]
                    )

            # three-term bf16 split of all staging rows, in column halves
            # so ACT and DVE pipeline instead of serializing full-width
            with tc.high_priority():
                for h in range(2):
                    hs = slice(h * (M // 2), (h + 1) * (M // 2))
                    nc.scalar.copy(BAA[:, hs], RCALL[:, hs])
                    nc.vector.tensor_tensor(
                        R1A[:, hs], RCALL[:, hs], BAA[:, hs], op=Alu.subtract
                    )
                    nc.scalar.copy(BBA[:, hs], R1A[:, hs])
                    nc.vector.tensor_tensor(
                        R2A[:, hs], R1A[:, hs], BBA[:, hs], op=Alu.subtract
                    )
                    nc.scalar.copy(BCA[:, hs], R2A[:, hs])

            # self coords partition-major for the t2 columns (finalize-only;
            # SWDGE queue; gated behind the splits via the tiny copy below so
            # its DRAM wires don't serialize ahead of the staging gathers)
            nc.gpsimd.dma_start(TP[0:1, :], BCA[0:1, 0 : NG * 2 * MT])
            for b in range(BPC):
                for s, (self_t, opp_t) in enumerate(((tgt, act), (act, tgt))):
                    g = 2 * b + s
                    nc.gpsimd.dma_start(
                        TP[:, g * 2 * MT : (g + 1) * 2 * MT].rearrange(
                            "p (i c) -> p i c", c=2
                        ),
                        self_t[b].rearrange("(i p) c -> p i c", p=128),
                    )
            nc.vector.tensor_mul(SQTP[:], TP[:], TP[:])
            SQv = SQTP[:].rearrange("p (k c) -> p k c", c=2)
            nc.vector.tensor_add(CS[:], SQv[:, :, 0], SQv[:, :, 1])

            # ---- main loop: K=15 bf16 matmuls + row-max reduce ----
            # Band scatter for group g is emitted just before g's units so
            # later groups' DMAs overlap the running compute; each group's 16
            # DMAs split across both HWDGE queues.
            NP = _CACHE.get("prelude", 0)  # fp32-prelude units in group 0
            for g in range(_CACHE.get("glimit", NG)):
                go = g ^ 1  # paired group: self splits of g = opp splits of go
                r = 32 * g
                c, co = 3 * g, 3 * go
                BAc, BBc, BCc = BAA, BBA, BCA
                BAo, BBo, BCo = BAA, BBA, BCA
                nc.sync.dma_start(R16[r + 0 : r + 2, :], BAc[c : c + 2, :])
                nc.sync.dma_start(R16[r + 2 : r + 4, :], BBc[c : c + 2, :])
                nc.sync.dma_start(R16[r + 4 : r + 6, :], BAc[c : c + 2, :])
                nc.sync.dma_start(R16[r + 6 : r + 8, :], BCc[c : c + 2, :])
                nc.sync.dma_start(R16[r + 8 : r + 10, :], BAc[c : c + 2, :])
                nc.sync.dma_start(R16[r + 10 : r + 12, :], BBc[c : c + 2, :])
                nc.sync.dma_start(R16[r + 12 : r + 13, :], BAc[c + 2 : c + 3, :])
                nc.sync.dma_start(R16[r + 13 : r + 14, :], BBc[c + 2 : c + 3, :])
                nc.sync.dma_start(R16[r + 14 : r + 15, :], BCc[c + 2 : c + 3, :])
                nc.sync.dma_start(L16[r + 0 : r + 2, :], BAo[co : co + 2, :])
                nc.sync.dma_start(L16[r + 2 : r + 4, :], BAo[co : co + 2, :])
                nc.sync.dma_start(L16[r + 4 : r + 6, :], BBo[co : co + 2, :])
                nc.sync.dma_start(L16[r + 6 : r + 8, :], BAo[co : co + 2, :])
                nc.sync.dma_start(L16[r + 8 : r + 10, :], BCo[co : co + 2, :])
                nc.sync.dma_start(L16[r + 10 : r + 12, :], BBo[co : co + 2, :])
                nc.sync.dma_start(L16[r + 12 : r + 15, :], CROWC3[:])
                lhsT = L16[32 * g : 32 * g + 15, :]
                rhs = R16[32 * g : 32 * g + 15, :]
                for i in range(_CACHE.get('mtlimit', MT)):
                    if g == 0 and i < NP:
                        lhsT_u, rhs_u = LF[:], RCALL[0:3, :]
                    else:
                        lhsT_u, rhs_u = lhsT, rhs
                    # Split PSUM halves into separate slot pools: the upper
                    # half recycles right after the ACT copy, so matmuls run
                    # two units ahead and the DVE scan stays back-to-back.
                    PU = psum_pool.tile([128, N // 2], fp32, tag="PU", bufs=2)
                    PL = psum_pool.tile([128, N // 2], fp32, tag="PL", bufs=2)
                    for j in range(2):
                        nc.tensor.matmul(
                            PU[:, 512 * j : 512 * (j + 1)],
                            lhsT_u[:, 128 * i : 128 * (i + 1)],
                            rhs_u[:, 512 * (j + 2) : 512 * (j + 3)],
                            start=True,
                            stop=True,
                            tile_position=(32 * g, 0),
                        )
                    # row-max via scan: ACT stages the upper half in SBUF so
                    # the DVE scan ingests 2 elems/cycle (PSUM + SBUF); the
                    # running max lands in the MR column via broadcast-out.
                    half = scr_pool.tile([128, N // 2], fp32, tag="half")
                    nc.scalar.copy(half[:], PU[:])
                    for j in range(2):
                        nc.tensor.matmul(
                            PL[:, 512 * j : 512 * (j + 1)],
                            lhsT_u[:, 128 * i : 128 * (i + 1)],
                            rhs_u[:, 512 * j : 512 * (j + 1)],
                            start=True,
                            stop=True,
                            tile_position=(32 * g, 0),
                        )
                    nc.vector.tensor_tensor_scan(
                        MR[:, g * MT + i : g * MT + i + 1].broadcast_to(
                            (128, N // 2)
                        ),
                        PL[:],
                        half[:],
                        initial=-3.0e38,
                        op0=Alu.max,
                        op1=Alu.max,
                    )

            # ---- finalize: d2min = CS - 2*MR ; d = sqrt(relu(d2min)) ----
            nc.vector.scalar_tensor_tensor(
                U[:], MR[:], -2.0, CS[:], op0=Alu.mult, op1=Alu.add
            )
            nc.vector.tensor_scalar_max(U[:], U[:], 0.0)
            nc.scalar.sqrt(D[:], U[:])
            nc.vector.reduce_sum(SUM[:], D[:], axis=mybir.AxisListType.X)
            # partition sum via PE: [1,1] = SUM.T @ ones
            ONES = singles.tile([128, 1], fp32, tag="ONES")
            OUTS = singles.tile([1, 1], fp32, tag="OUTS")
            nc.gpsimd.memset(ONES[:], 1.0)
            acc = psum_pool.tile([1, 1], fp32, tag="PL", bufs=2)
            nc.tensor.matmul(acc[:], SUM[:], ONES[:], start=True, stop=True)
            nc.scalar.copy(OUTS[:], acc[:])
            nc.sync.dma_start(out, OUTS[:])

    nc.compile()
    return nc


def _get_program():
    if "nc" not in _CACHE:
        _CACHE["nc"] = _build_program()
    return _CACHE["nc"]


def kernel(target_points, actual_points):
    tgt = np.ascontiguousarray(np.asarray(target_points, dtype=np.float32))
    act = np.ascontiguousarray(np.asarray(actual_points, dtype=np.float32))
    assert tgt.shape == (B, M, 2) and act.shape == (B, N, 2)

    nc = _get_program()
    in_maps = [
        {"tgt": tgt[c * BPC : (c + 1) * BPC], "act": act[c * BPC : (c + 1) * BPC]}
        for c in range(NCORES)
    ]
    from concourse import bass_utils

    res = bass_utils.run_bass_kernel_spmd(nc, in_maps, core_ids=list(range(NCORES)))
    total = sum(float(r["out"][0, 0]) for r in res.results)
    return np.float32(total / (B * M))



# revision 20
# speedup vs baseline: 1.0463x; 1.0157x over previous
"""Chamfer loss kernel for Trainium2 (8 NeuronCores, SPMD).

Math: for each batch b, d2[m,n] = ||t[m]-a[n]||^2 over 2D points.
  loss = mean_{b,m} min_n d[m,n] + mean_{b,n} min_m d[m,n]

Strategy per core (2 batches/core, data-parallel over batch):
  - P[m,n] = t.a - a2[n]/2 computed on the TensorEngine, so
    min_n d2 = t2[m] - 2*max_n P[m,n]  (t2 applied per-partition afterwards).
  - fp32 matmuls run at 4 cycles/row on TRN2; instead each value is split
    into THREE bf16 terms (t = t1+t2+t3, a = a1+a2+a3, s2 = q1+q2+q3) and
    P is ONE K=15 bf16 matmul (1 cycle/row) keeping all cross products
    t_i.a_j with i+j <= 4, giving ~1e-6 absolute d2 error (2-term splits
    leave ~1e-4 tails, the same order as d2min itself).
  - Row-max of each [128,2048] PSUM tile via DVE reduce_max (the fused
    tensor_tensor_reduce ISA op crashes this runtime).
  - Backward direction = same kernel with roles swapped (second pass).
  - Finalize: d = sqrt(relu(-2*max + s2col)), sum on-chip, PE ones-matmul
    partition-sum, one scalar out per core; host sums 8 scalars and divides.

Groups g = 2*b + s, s=0: fwd (self=target, opp=actual), s=1: bwd (swapped).
Group g's matmul operands live at partitions 32g..32g+8 (PE row-group g).
"""

import numpy as np

B, M, N = 16, 2048, 2048
NCORES = 8
BPC = B // NCORES  # batches per core
NG = 2 * BPC  # groups per core: (fwd,bwd) x batches
MT = M // 128  # m-tiles per group

_CACHE = {}


def _build_program():
    from concourse import bacc, mybir

    fp32 = mybir.dt.float32
    bf16 = mybir.dt.bfloat16
    Alu = mybir.AluOpType
    import concourse.tile as tile

    nc = bacc.Bacc("TRN2", target_bir_lowering=False, debug=False)
    tgt = nc.dram_tensor("tgt", [BPC, M, 2], fp32, kind="ExternalInput").ap()
    act = nc.dram_tensor("act", [BPC, M, 2], fp32, kind="ExternalInput").ap()
    out = nc.dram_tensor("out", [1, 1], fp32, kind="ExternalOutput").ap()

    with tile.TileContext(nc) as tc:
        with (
            tc.tile_pool(name="singles", bufs=1) as singles,
            tc.tile_pool(name="scr", bufs=3) as scr_pool,
            tc.tile_pool(name="psum", bufs=2, space="PSUM") as psum_pool,
        ):
            DUM = singles.tile([1, 1], fp32, tag="DUM")
            nc.gpsimd.memset(DUM[:], 1.0)
            nc.scalar.square(DUM[:], DUM[:])
            # bf16 matmul operand tiles; group g occupies rows 32g..32g+14.
            # K=15 band pairing (lhsT row . rhs row):
            #   L1x.A1x L1y.A1y | L1x.A2x L1y.A2y | L2x.A1x L2y.A1y
            #   L1x.A3x L1y.A3y | L3x.A1x L3y.A1y | L2x.A2x L2y.A2y
            #   c.q1 c.q2 c.q3   (c = -1/2, q_i = s2 splits)
            L16 = singles.tile([128, M], bf16, tag="L16")
            R16 = singles.tile([128, M], bf16, tag="R16")
            # fp32 staging rows 3g+{0,1,2} = [x_opp, y_opp, s2_opp], base 0
            RCALL = singles.tile([3 * NG, M], fp32, tag="RCALL")
            BAA = singles.tile([3 * NG, M], bf16, tag="BAA")  # term 1
            BBA = singles.tile([3 * NG, M], bf16, tag="BBA")  # term 2
            BCA = singles.tile([3 * NG, M], bf16, tag="BCA")  # term 3
            R1A = singles.tile([3 * NG, M], fp32, tag="R1A")
            R2A = singles.tile([3 * NG, M], fp32, tag="R2A")
            # opp coords for squares (x block, y block), contiguous base 0
            XYX = singles.tile([NG, M], fp32, tag="XYX")
            XYY = singles.tile([NG, M], fp32, tag="XYY")
            SQX = singles.tile([NG, M], fp32, tag="SQX")
            SQY = singles.tile([NG, M], fp32, tag="SQY")
            S2 = singles.tile([NG, M], fp32, tag="S2")
            TP = singles.tile([128, NG * 2 * MT], fp32, tag="TP")
            SQTP = singles.tile([128, NG * 2 * MT], fp32, tag="SQTP")
            CS = singles.tile([128, NG * MT], fp32, tag="CS")
            MR = singles.tile([128, NG * MT], fp32, tag="MR")
            U = singles.tile([128, NG * MT], fp32, tag="U")
            D = singles.tile([128, NG * MT], fp32, tag="D")
            SUM = singles.tile([128, 1], fp32, tag="SUM")
            CROWC3 = singles.tile([3, M], bf16, tag="CROWC3")

            # ---- input staging ----
            for b in range(BPC):
                for s, (self_t, opp_t) in enumerate(((tgt, act), (act, tgt))):
                    g = 2 * b + s
                    om = opp_t[b].rearrange("m c -> c m")  # [2, M] coord-major
                    eng = nc.sync if g % 2 == 0 else nc.scalar
                    eng.dma_start(RCALL[3 * g : 3 * g + 2, :], om)
                    nc.gpsimd.dma_start(XYX[g : g + 1, :], RCALL[3 * g : 3 * g + 1, :])
                    nc.gpsimd.dma_start(
                        XYY[g : g + 1, :], RCALL[3 * g + 1 : 3 * g + 2, :]
                    )

            # fp32 prelude operands for group 0's first units: lhsT rows
            # [x_T0, y_T0, -1/2] (exact fp32, no split-chain dependency)
            LF = singles.tile([3, M], fp32, tag="LF")
            CROWCF = singles.tile([1, M], fp32, tag="CROWCF")
            nc.gpsimd.memset(CROWCF[:], -0.5)
            nc.sync.dma_start(LF[0:2, :], RCALL[3:5, :])
            nc.sync.dma_start(LF[2:3, :], CROWCF[:])

            nc.gpsimd.memset(CROWC3[:], -0.5)
            # opp squared norms: S2[g] = x^2 + y^2, by column halves so the
            # split chain's first half unblocks sooner
            for h in range(2):
                hs = slice(h * (M // 2), (h + 1) * (M // 2))
                nc.scalar.square(SQX[:, hs], XYX[:, hs])
                nc.vector.tensor_mul(SQY[:, hs], XYY[:, hs], XYY[:, hs])
                nc.vector.tensor_add(S2[:, hs], SQX[:, hs], SQY[:, hs])
                for g in range(NG):
                    eng = nc.sync if g % 2 == 0 else nc.scalar
                    eng.dma_start(
                        RCALL[3 * g + 2 : 3 * g + 3, hs], S2[g : g + 1, hs]
                    )

            # three-term bf16 split of all staging rows, in column halves
            # so ACT and DVE pipeline instead of serializing full-width
            with tc.high_priority():
                for h in range(2):
                    hs = slice(h * (M // 2), (h + 1) * (M // 2))
                    nc.scalar.copy(BAA[:, hs], RCALL[:, hs])
                    nc.vector.tensor_tensor(
                        R1A[:, hs], RCALL[:, hs], BAA[:, hs], op=Alu.subtract
                    )
                    nc.scalar.copy(BBA[:, hs], R1A[:, hs])
                    nc.vector.tensor_tensor(
                        R2A[:, hs], R1A[:, hs], BBA[:, hs], op=Alu.subtract
                    )
                    nc.scalar.copy(BCA[:, hs], R2A[:, hs])

            # self coords partition-major for the t2 columns (finalize-only;
            # SWDGE queue; gated behind the splits via the tiny copy below so
            # its DRAM wires don't serialize ahead of the staging gathers)
            nc.gpsimd.dma_start(TP[0:1, :], BCA[0:1, 0 : NG * 2 * MT])
            for b in range(BPC):
                for s, (self_t, opp_t) in enumerate(((tgt, act), (act, tgt))):
                    g = 2 * b + s
                    nc.gpsimd.dma_start(
                        TP[:, g * 2 * MT : (g + 1) * 2 * MT].rearrange(
                            "p (i c) -> p i c", c=2
                        ),
                        self_t[b].rearrange("(i p) c -> p i c", p=128),
                    )
            nc.vector.tensor_mul(SQTP[:], TP[:], TP[:])
            SQv = SQTP[:].rearrange("p (k c) -> p k c", c=2)
            nc.vector.tensor_add(CS[:], SQv[:, :, 0], SQv[:, :, 1])

            # ---- main loop: K=15 bf16 matmuls + row-max reduce ----
            # Band scatter for group g is emitted just before g's units so
            # later groups' DMAs overlap the running compute; each group's 16
            # DMAs split across both HWDGE queues.
            NP = _CACHE.get("prelude", 0)  # fp32-prelude units in group 0
            for g in range(_CACHE.get("glimit", NG)):
                go = g ^ 1  # paired group: self splits of g = opp splits of go
                r = 32 * g
                c, co = 3 * g, 3 * go
                BAc, BBc, BCc = BAA, BBA, BCA
                BAo, BBo, BCo = BAA, BBA, BCA
                nc.sync.dma_start(R16[r + 0 : r + 2, :], BAc[c : c + 2, :])
                nc.sync.dma_start(R16[r + 2 : r + 4, :], BBc[c : c + 2, :])
                nc.sync.dma_start(R16[r + 4 : r + 6, :], BAc[c : c + 2, :])
                nc.sync.dma_start(R16[r + 6 : r + 8, :], BCc[c : c + 2, :])
                nc.sync.dma_start(R16[r + 8 : r + 10, :], BAc[c : c + 2, :])
                nc.sync.dma_start(R16[r + 10 : r + 12, :], BBc[c : c + 2, :])
                nc.sync.dma_start(R16[r + 12 : r + 13, :], BAc[c + 2 : c + 3, :])
                nc.sync.dma_start(R16[r + 13 : r + 14, :], BBc[c + 2 : c + 3, :])
                nc.sync.dma_start(R16[r + 14 : r + 15, :], BCc[c + 2 : c + 3, :])
                nc.sync.dma_start(L16[r + 0 : r + 2, :], BAo[co : co + 2, :])
                nc.sync.dma_start(L16[r + 2 : r + 4, :], BAo[co : co + 2, :])
                nc.sync.dma_start(L16[r + 4 : r + 6, :], BBo[co : co + 2, :])
                nc.sync.dma_start(L16[r + 6 : r + 8, :], BAo[co : co + 2, :])
                nc.sync.dma_start(L16[r + 8 : r + 10, :], BCo[co : co + 2, :])
                nc.sync.dma_start(L16[r + 10 : r + 12, :], BBo[co : co + 2, :])
                nc.sync.dma_start(L16[r + 12 : r + 15, :], CROWC3[:])
                lhsT = L16[32 * g : 32 * g + 15, :]
                rhs = R16[32 * g : 32 * g + 15, :]
                for i in range(_CACHE.get('mtlimit', MT)):
                    if g == 0 and i < NP:
                        lhsT_u, rhs_u = LF[:], RCALL[0:3, :]
                    else:
                        lhsT_u, rhs_u = lhsT, rhs
                    # Split PSUM halves into separate slot pools: the upper
                    # half recycles right after the ACT copy, so matmuls run
                    # two units ahead and the DVE scan stays back-to-back.
                    PU = psum_pool.tile([128, N // 2], fp32, tag="PU", bufs=2)
                    PL = psum_pool.tile([128, N // 2], fp32, tag="PL", bufs=2)
                    for j in range(2):
                        nc.tensor.matmul(
                            PU[:, 512 * j : 512 * (j + 1)],
                            lhsT_u[:, 128 * i : 128 * (i + 1)],
                            rhs_u[:, 512 * (j + 2) : 512 * (j + 3)],
                            start=True,
                            stop=True,
                            tile_position=(32 * g, 0),
                        )
                    # row-max via scan: ACT stages the upper half in SBUF so
                    # the DVE scan ingests 2 elems/cycle (PSUM + SBUF); the
                    # running max lands in the MR column via broadcast-out.
                    half = scr_pool.tile([128, N // 2], fp32, tag="half")
                    nc.scalar.copy(half[:], PU[:])
                    for j in range(2):
                        nc.tensor.matmul(
                            PL[:, 512 * j : 512 * (j + 1)],
                            lhsT_u[:, 128 * i : 128 * (i + 1)],
                            rhs_u[:, 512 * j : 512 * (j + 1)],
                            start=True,
                            stop=True,
                            tile_position=(32 * g, 0),
                        )
                    nc.vector.tensor_tensor_scan(
                        MR[:, g * MT + i : g * MT + i + 1].broadcast_to(
                            (128, N // 2)
                        ),
                        PL[:],
                        half[:],
                        initial=-3.0e38,
                        op0=Alu.max,
                        op1=Alu.max,
                    )

            # ---- finalize: d2min = CS - 2*MR ; d = sqrt(relu(d2min)) ----
            nc.vector.scalar_tensor_tensor(
                U[:], MR[:], -2.0, CS[:], op0=Alu.mult, op1=Alu.add
            )
            nc.vector.tensor_scalar_max(U[:], U[:], 0.0)
            nc.scalar.sqrt(D[:], U[:])
            nc.vector.reduce_sum(SUM[:], D[:], axis=mybir.AxisListType.X)
            # partition sum via PE: [1,1] = SUM.T @ ones
            ONES = singles.tile([128, 1], fp32, tag="ONES")
            OUTS = singles.tile([1, 1], fp32, tag="OUTS")
            nc.gpsimd.memset(ONES[:], 1.0)
            acc = psum_pool.tile([1, 1], fp32, tag="PL", bufs=2)
            nc.tensor.matmul(acc[:], SUM[:], ONES[:], start=True, stop=True)
            nc.scalar.copy(OUTS[:], acc[:])
            nc.sync.dma_start(out, OUTS[:])

    nc.compile()
    return nc


def _get_program():
    if "nc" not in _CACHE:
        _CACHE["nc"] = _build_program()
    return _CACHE["nc"]


def kernel(target_points, actual_points):
    tgt = np.ascontiguousarray(np.asarray(target_points, dtype=np.float32))
    act = np.ascontiguousarray(np.asarray(actual_points, dtype=np.float32))
    assert tgt.shape == (B, M, 2) and act.shape == (B, N, 2)

    nc = _get_program()
    in_maps = [
        {"tgt": tgt[c * BPC : (c + 1) * BPC], "act": act[c * BPC : (c + 1) * BPC]}
        for c in range(NCORES)
    ]
    from concourse import bass_utils

    res = bass_utils.run_bass_kernel_spmd(nc, in_maps, core_ids=list(range(NCORES)))
    total = sum(float(r["out"][0, 0]) for r in res.results)
    return np.float32(total / (B * M))

